# revision 16
# baseline (speedup 1.0000x reference)
"""CorrelationLayer1D Trainium2 Bass kernel (v6).

Computes out[b, d, h, w] = sum_c x_1[b,c,h,w] * x2p[b,c,h,w+d] for d in [0, 41),
where x2p is x_2 width-padded by (8, 32).  Inputs [4,128,160,320] f32.

Sharding: data-parallel over H = 160 = 8*20 (correlation runs along W only, so
H-sharding needs no halo).  Per core: chunks of HC=10 rows, row-pairs NG=2.

Key structure (per (b, chunk)):
  - fp32->bf16 casting loads (SWDGE/gpsimd), contiguous [C, hc*W].
  - Grams: per row, 3 matmuls with M=128/128/64 x1 stationaries against
    clipped x2 windows (edge pads realized by narrowing + atlas memsets).
    Each row's matmul writes PSUM with free stride NG=2 (8B), interleaving
    the row-pair j-major: the group tile holds [M, j*2 + q].  Strided PSUM
    drain was HW-measured to cost ~0 extra.
  - The atlas keeps the FULL j in [0, M+40) window per block (no 64-half
    split), so the band shear j = p + d is uniform over all 128 partitions:
    scr addr = p*(rowpitch + 2) + group*2*nw + (d*2 + q).  Atlas copies are
    pure rectangles; blocks 0+1 share one [128, 2*hc*168] tile.
  - DMA instructions per chunk: 2 casting loads, 2 rect scratch stores
    (blk01, blk2), 3 skewed reloads (one per block, groups merged into a
    3-dim AP, (d,q)-contiguous 164B runs, 128 descriptors), 5 output
    stores.  Issuing-queue occupancy was the v5 bottleneck at 21 instrs.
  - Back-end (emitted after the NEXT chunk's front so the PE never waits on
    the scratch round trip): per (row-pair, block) one PE transpose of the
    contiguous [M, 2*D] sbig slice -> [2*41, M] PSUM (partition = (d, q)),
    one cast copy into the row-pair's [82, W] f32 tile, then one store
    straight to out[b, :, h0+2g : h0+2g+2, :] via a 3-dim affine DRAM AP.
"""

import sys

import numpy as np

try:
    import concourse.bass as bass  # noqa: F401
except ImportError:
    sys.path.insert(0, "/opt/trn_rl_repo")

import concourse.bass as bass
import concourse.tile as tile
from concourse import bacc, masks, mybir
from concourse.ap import AP
from concourse.bass_utils import run_bass_kernel_spmd

MAX_DISP = 40
D = MAX_DISP + 1  # 41 displacements
PAD_L = 8
PAD_R = 32
B, C, H, W = 4, 128, 160, 320
N_CORES = 8
HS = H // N_CORES  # 20 h-rows per core
WBLOCKS = [(0, 128), (128, 128), (256, 64)]  # (w0, M); window width nw = M + 40

F32 = mybir.dt.float32
BF16 = mybir.dt.bfloat16


def _clip(w0, M):
    """Window [w0-8, w0+M+32) clipped to [0, W): (x2_lo_col, jlo, jhi)."""
    lo = max(w0 - PAD_L, 0)
    hi = min(w0 + M + PAD_R, W)
    jlo = lo - (w0 - PAD_L)
    return lo, jlo, jlo + (hi - lo)


def build_kernel(b_dim=B, hs=HS, hc=10, xin_bufs=4):
    ng = 2
    assert hs % hc == 0 and hc % ng == 0
    nchunks = hs // hc
    ngrp = hc // ng
    NW0 = 168  # blocks 0/1 window width
    NW2 = 104  # block 2 window width
    P01 = 2 * hc * NW0  # atl01/scr01 row pitch in elements (two kb blocks)

    nc = bacc.Bacc("TRN2", target_bir_lowering=False, debug=False)
    x1e = nc.declare_dram_parameter("x1", [b_dim, C, hs, W], F32, isOutput=False)
    x2e = nc.declare_dram_parameter("x2", [b_dim, C, hs, W], F32, isOutput=False)
    oute = nc.declare_dram_parameter("out", [b_dim, D, hs, W], F32, isOutput=True)

    with tile.TileContext(nc) as tc:
        with (
            tc.tile_pool(name="const", bufs=1) as const_pool,
            tc.tile_pool(name="xin", bufs=xin_bufs) as xin_pool,
            tc.tile_pool(name="atlas", bufs=2) as atlas_pool,
            tc.tile_pool(name="sbig", bufs=2) as sbig_pool,
            tc.tile_pool(name="asm", bufs=2) as asm_pool,
            tc.tile_pool(name="psum_g", bufs=2, space="PSUM") as psum_g,
            tc.tile_pool(name="psum_t", bufs=2, space="PSUM") as psum_t,
            tc.tile_pool(name="scratch", bufs=2, space="DRAM") as scratch_pool,
        ):
            identity = const_pool.tile([128, 128], BF16)
            masks.make_identity(nc, identity[:])

            def emit_front(b, ci):
                h0 = ci * hc
                x1b = xin_pool.tile(
                    [C, hc * W], BF16, tag="x1b", name=f"x1b_{b}_{ci}"
                )
                nc.gpsimd.dma_start(
                    x1b[:].rearrange("p (h w) -> p h w", w=W),
                    x1e[b, :, h0 : h0 + hc, :],
                )
                x2b = xin_pool.tile(
                    [C, hc * W], BF16, tag="x2b", name=f"x2b_{b}_{ci}"
                )
                nc.gpsimd.dma_start(
                    x2b[:].rearrange("p (h w) -> p h w", w=W),
                    x2e[b, :, h0 : h0 + hc, :],
                )

                # Atlas: blocks 0+1 in one [128, P01] tile (kb-region pitch
                # hc*NW0), block 2 in [64, hc*NW2]; both group-blocked with
                # the row-pair interleaved: col = kb_off + g*(ng*nw) + j*ng + q.
                atl01 = atlas_pool.tile(
                    [128, P01], BF16, tag="A01", name=f"A01_{b}_{ci}"
                )
                atl2 = atlas_pool.tile(
                    [64, hc * NW2], BF16, tag="A2", name=f"A2_{b}_{ci}"
                )
                # Zero the j-columns the narrowed edge windows skip:
                # block 0: j in [0, 8); block 2: j in [72, 104).  One strided
                # memset each, spanning all groups.
                a01v = atl01[:].rearrange("p (g j) -> p g j", j=ng * NW0)
                nc.gpsimd.memset(a01v[:, 0:ngrp, 0 : ng * PAD_L], 0.0)
                a2v = atl2[:].rearrange("p (g j) -> p g j", j=ng * NW2)
                nc.gpsimd.memset(a2v[:, :, ng * 72 : ng * NW2], 0.0)

                ncop = 0
                for gi in range(ngrp):
                    g0 = gi * ng
                    ps = [
                        psum_g.tile(
                            [M, ng * (M + MAX_DISP)],
                            F32,
                            tag=f"g{kb}",
                            name=f"g{kb}_{b}_{ci}_{gi}",
                        )
                        for kb, (w0, M) in enumerate(WBLOCKS)
                    ]
                    for q in range(ng):
                        hh = g0 + q
                        for kb, (w0, M) in enumerate(WBLOCKS):
                            lo, jlo, jhi = _clip(w0, M)
                            pb = ps[kb][:]
                            out_ap = AP(
                                tensor=pb.tensor,
                                offset=pb.offset + ng * jlo + q,
                                ap=[list(pb.ap[0]), [ng, jhi - jlo]],
                            )
                            nc.tensor.matmul(
                                out_ap,
                                x1b[:, hh * W + w0 : hh * W + w0 + M],
                                x2b[:, hh * W + lo : hh * W + lo + (jhi - jlo)],
                                start=True,
                                stop=True,
                            )
                    for kb, (w0, M) in enumerate(WBLOCKS):
                        nw = M + MAX_DISP
                        lo, jlo, jhi = _clip(w0, M)
                        src = ps[kb][0:M, ng * jlo : ng * jhi]
                        if kb < 2:
                            base = kb * hc * NW0 + gi * ng * NW0
                            dst = atl01[:, base + ng * jlo : base + ng * jhi]
                        else:
                            base = gi * ng * NW2
                            dst = atl2[:, base + ng * jlo : base + ng * jhi]
                        if ncop % 5 < 3:
                            nc.vector.tensor_copy(dst, src)
                        else:
                            nc.scalar.copy(dst, src)
                        ncop += 1

                # Big stores ride SWDGE (async descriptor-gen); the sync
                # queue blocks ~transfer-time per HWDGE DMA instruction.
                scr01 = scratch_pool.tile(
                    [128, P01], BF16, tag="scr01", name=f"scr01_{b}_{ci}"
                )
                nc.gpsimd.dma_start(scr01[:], atl01[:])
                scr2 = scratch_pool.tile(
                    [64, hc * NW2], BF16, tag="scr2", name=f"scr2_{b}_{ci}"
                )
                nc.gpsimd.dma_start(scr2[:], atl2[:])

                # Skewed band reloads: band j = p + d, so
                # addr = p*(pitch + ng) + kb_off + g*(ng*nw) + (d*ng + q).
                sbig = []
                for kb, (w0, M) in enumerate(WBLOCKS):
                    sb = sbig_pool.tile(
                        [M, hc * D], BF16, tag=f"sb{kb}", name=f"sb{kb}_{b}_{ci}"
                    )
                    if kb < 2:
                        sap = scr01[:]
                        dims = [
                            [P01 + ng, 128],
                            [ng * NW0, ngrp],
                            [1, ng * D],
                        ]
                        off = sap.offset + kb * hc * NW0
                    else:
                        sap = scr2[:]
                        dims = [
                            [hc * NW2 + ng, 64],
                            [ng * NW2, ngrp],
                            [1, ng * D],
                        ]
                        off = sap.offset
                    diag = AP(tensor=sap.tensor, offset=off, ap=dims)
                    nc.scalar.dma_start(sb[:], diag)
                    sbig.append(sb)
                return sbig

            def emit_back(b, ci, sbig):
                h0 = ci * hc
                ncop = 0
                for gi in range(hc // ng):
                    asm = asm_pool.tile(
                        [ng * D, W], F32, tag=f"as{gi}", name=f"as{gi}_{b}_{ci}"
                    )
                    for kb, (w0, M) in enumerate(WBLOCKS):
                        t_ps = psum_t.tile(
                            [ng * D, 128],
                            BF16,
                            tag="t_ps",
                            name=f"t_ps_{b}_{ci}_{gi}_{kb}",
                        )
                        nc.tensor.matmul(
                            t_ps[:, 0:M],
                            sbig[kb][0:M, gi * ng * D : (gi + 1) * ng * D],
                            identity[0:M, 0:M],
                            start=True,
                            stop=True,
                            is_transpose=True,
                        )
                        dst = asm[:, w0 : w0 + M]
                        if ncop % 5 < 3:
                            nc.vector.tensor_copy(dst, t_ps[:, 0:M])
                        else:
                            nc.scalar.copy(dst, t_ps[:, 0:M])
                        ncop += 1
                    # Partition p = d*ng + q -> out[b, d, h0+gi*ng+q, :].
                    ob = oute[b, 0:D, h0 + gi * ng : h0 + gi * ng + ng, :]
                    nc.sync.dma_start(ob, asm[:])

            prev = None
            for b in range(b_dim):
                for ci in range(nchunks):
                    sbig = emit_front(b, ci)
                    if prev is not None:
                        emit_back(*prev)
                    prev = (b, ci, sbig)
            emit_back(*prev)

    nc.finalize()
    return nc


_compiled = {}


def _get_kernel(b_dim, hs):
    key = (b_dim, hs)
    if key not in _compiled:
        _compiled[key] = build_kernel(b_dim, hs)
    return _compiled[key]


def kernel(x_1: np.ndarray, x_2: np.ndarray) -> np.ndarray:
    assert x_1.shape == (B, C, H, W) and x_2.shape == (B, C, H, W)
    x_1 = np.ascontiguousarray(x_1, dtype=np.float32)
    x_2 = np.ascontiguousarray(x_2, dtype=np.float32)
    nc = _get_kernel(B, HS)
    in_maps = [
        {
            "x1": np.ascontiguousarray(x_1[:, :, i * HS : (i + 1) * HS, :]),
            "x2": np.ascontiguousarray(x_2[:, :, i * HS : (i + 1) * HS, :]),
        }
        for i in range(N_CORES)
    ]
    res = run_bass_kernel_spmd(nc, in_maps, core_ids=list(range(N_CORES))).results
    out = np.concatenate([res[i]["out"] for i in range(N_CORES)], axis=2)
    return out


# revision 19
# speedup vs baseline: 1.3345x; 1.3345x over previous
"""CorrelationLayer1D Trainium2 Bass kernel (v6).

Computes out[b, d, h, w] = sum_c x_1[b,c,h,w] * x2p[b,c,h,w+d] for d in [0, 41),
where x2p is x_2 width-padded by (8, 32).  Inputs [4,128,160,320] f32.

Sharding: data-parallel over H = 160 = 8*20 (correlation runs along W only, so
H-sharding needs no halo).  Per core: chunks of HC=10 rows, row-pairs NG=2.

Key structure (per (b, chunk)):
  - fp32->bf16 casting loads (SWDGE/gpsimd), contiguous [C, hc*W].
  - Grams: per row, 3 matmuls with M=128/128/64 x1 stationaries against
    clipped x2 windows (edge pads realized by narrowing + atlas memsets).
    Each row's matmul writes PSUM with free stride NG=2 (8B), interleaving
    the row-pair j-major: the group tile holds [M, j*2 + q].  Strided PSUM
    drain was HW-measured to cost ~0 extra.
  - The atlas keeps the FULL j in [0, M+40) window per block (no 64-half
    split), so the band shear j = p + d is uniform over all 128 partitions:
    scr addr = p*(rowpitch + 2) + group*2*nw + (d*2 + q).  Atlas copies are
    pure rectangles; blocks 0+1 share one [128, 2*hc*168] tile.
  - DMA instructions per chunk: 2 casting loads, 2 rect scratch stores
    (blk01, blk2), 3 skewed reloads (one per block, groups merged into a
    3-dim AP, (d,q)-contiguous 164B runs, 128 descriptors), 5 output
    stores.  Issuing-queue occupancy was the v5 bottleneck at 21 instrs.
  - Back-end (emitted after the NEXT chunk's front so the PE never waits on
    the scratch round trip): per (row-pair, block) one PE transpose of the
    contiguous [M, 2*D] sbig slice -> [2*41, M] PSUM (partition = (d, q)),
    one cast copy into the row-pair's [82, W] f32 tile, then one store
    straight to out[b, :, h0+2g : h0+2g+2, :] via a 3-dim affine DRAM AP.
"""

import sys

import numpy as np

try:
    import concourse.bass as bass  # noqa: F401
except ImportError:
    sys.path.insert(0, "/opt/trn_rl_repo")

import concourse.bass as bass
import concourse.tile as tile
from concourse import bacc, masks, mybir
from concourse.ap import AP
from concourse.bass_utils import run_bass_kernel_spmd

MAX_DISP = 40
D = MAX_DISP + 1  # 41 displacements
PAD_L = 8
PAD_R = 32
B, C, H, W = 4, 128, 160, 320
N_CORES = 8
HS = H // N_CORES  # 20 h-rows per core
WBLOCKS = [(0, 128), (128, 128), (256, 64)]  # (w0, M); window width nw = M + 40

F32 = mybir.dt.float32
BF16 = mybir.dt.bfloat16


def _clip(w0, M):
    """Window [w0-8, w0+M+32) clipped to [0, W): (x2_lo_col, jlo, jhi)."""
    lo = max(w0 - PAD_L, 0)
    hi = min(w0 + M + PAD_R, W)
    jlo = lo - (w0 - PAD_L)
    return lo, jlo, jlo + (hi - lo)


def build_kernel(b_dim=B, hs=HS, hc=10, xin_bufs=4):
    ng = 2
    assert hs % hc == 0 and hc % ng == 0
    nchunks = hs // hc
    ngrp = hc // ng
    NW0 = 168  # blocks 0/1 window width
    NW2 = 104  # block 2 window width
    P01 = 2 * hc * NW0  # atl01/scr01 row pitch in elements (two kb blocks)

    nc = bacc.Bacc("TRN2", target_bir_lowering=False, debug=False)
    x1e = nc.declare_dram_parameter("x1", [b_dim, C, hs, W], F32, isOutput=False)
    x2e = nc.declare_dram_parameter("x2", [b_dim, C, hs, W], F32, isOutput=False)
    oute = nc.declare_dram_parameter("out", [b_dim, D, hs, W], F32, isOutput=True)

    with tile.TileContext(nc) as tc:
        with (
            tc.tile_pool(name="const", bufs=1) as const_pool,
            tc.tile_pool(name="xin", bufs=xin_bufs) as xin_pool,
            tc.tile_pool(name="atlas", bufs=2) as atlas_pool,
            tc.tile_pool(name="sbig", bufs=2) as sbig_pool,
            tc.tile_pool(name="asm", bufs=2) as asm_pool,
            tc.tile_pool(name="psum_g", bufs=2, space="PSUM") as psum_g,
            tc.tile_pool(name="psum_t", bufs=2, space="PSUM") as psum_t,
            tc.tile_pool(name="scratch", bufs=2, space="DRAM") as scratch_pool,
        ):
            identity = const_pool.tile([128, 128], BF16)
            masks.make_identity(nc, identity[:])

            def emit_front(b, ci):
                h0 = ci * hc
                x1b = xin_pool.tile(
                    [C, hc * W], BF16, tag="x1b", name=f"x1b_{b}_{ci}"
                )
                nc.gpsimd.dma_start(
                    x1b[:].rearrange("p (h w) -> p h w", w=W),
                    x1e[b, :, h0 : h0 + hc, :],
                )
                x2b = xin_pool.tile(
                    [C, hc * W], BF16, tag="x2b", name=f"x2b_{b}_{ci}"
                )
                nc.gpsimd.dma_start(
                    x2b[:].rearrange("p (h w) -> p h w", w=W),
                    x2e[b, :, h0 : h0 + hc, :],
                )

                # Atlas: blocks 0+1 in one [128, P01] tile (kb-region pitch
                # hc*NW0), block 2 in [64, hc*NW2]; both group-blocked with
                # the row-pair interleaved: col = kb_off + g*(ng*nw) + j*ng + q.
                atl01 = atlas_pool.tile(
                    [128, P01], BF16, tag="A01", name=f"A01_{b}_{ci}"
                )
                atl2 = atlas_pool.tile(
                    [64, hc * NW2], BF16, tag="A2", name=f"A2_{b}_{ci}"
                )
                # Zero the j-columns the narrowed edge windows skip:
                # block 0: j in [0, 8); block 2: j in [72, 104).  One strided
                # memset each, spanning all groups.
                a01v = atl01[:].rearrange("p (g j) -> p g j", j=ng * NW0)
                nc.gpsimd.memset(a01v[:, 0:ngrp, 0 : ng * PAD_L], 0.0)
                a2v = atl2[:].rearrange("p (g j) -> p g j", j=ng * NW2)
                nc.gpsimd.memset(a2v[:, :, ng * 72 : ng * NW2], 0.0)

                ncop = 0
                for gi in range(ngrp):
                    g0 = gi * ng
                    ps = [
                        psum_g.tile(
                            [M, ng * (M + MAX_DISP)],
                            F32,
                            tag=f"g{kb}",
                            name=f"g{kb}_{b}_{ci}_{gi}",
                        )
                        for kb, (w0, M) in enumerate(WBLOCKS)
                    ]
                    for q in range(ng):
                        hh = g0 + q
                        for kb, (w0, M) in enumerate(WBLOCKS):
                            lo, jlo, jhi = _clip(w0, M)
                            pb = ps[kb][:]
                            out_ap = AP(
                                tensor=pb.tensor,
                                offset=pb.offset + ng * jlo + q,
                                ap=[list(pb.ap[0]), [ng, jhi - jlo]],
                            )
                            nc.tensor.matmul(
                                out_ap,
                                x1b[:, hh * W + w0 : hh * W + w0 + M],
                                x2b[:, hh * W + lo : hh * W + lo + (jhi - jlo)],
                                start=True,
                                stop=True,
                            )
                    for kb, (w0, M) in enumerate(WBLOCKS):
                        nw = M + MAX_DISP
                        lo, jlo, jhi = _clip(w0, M)
                        src = ps[kb][0:M, ng * jlo : ng * jhi]
                        if kb < 2:
                            base = kb * hc * NW0 + gi * ng * NW0
                            dst = atl01[:, base + ng * jlo : base + ng * jhi]
                        else:
                            base = gi * ng * NW2
                            dst = atl2[:, base + ng * jlo : base + ng * jhi]
                        if ncop % 5 < 3:
                            nc.vector.tensor_copy(dst, src)
                        else:
                            nc.scalar.copy(dst, src)
                        ncop += 1

                # Big stores ride SWDGE (async descriptor-gen); the sync
                # queue blocks ~transfer-time per HWDGE DMA instruction.
                scr01 = scratch_pool.tile(
                    [128, P01], BF16, tag="scr01", name=f"scr01_{b}_{ci}"
                )
                nc.sync.dma_start(scr01[:], atl01[:])
                scr2 = scratch_pool.tile(
                    [64, hc * NW2], BF16, tag="scr2", name=f"scr2_{b}_{ci}"
                )
                nc.sync.dma_start(scr2[:], atl2[:])

                # Skewed band reloads: band j = p + d, so
                # addr = p*(pitch + ng) + kb_off + g*(ng*nw) + (d*ng + q).
                sbig = []
                for kb, (w0, M) in enumerate(WBLOCKS):
                    sb = sbig_pool.tile(
                        [M, hc * D], BF16, tag=f"sb{kb}", name=f"sb{kb}_{b}_{ci}"
                    )
                    if kb < 2:
                        sap = scr01[:]
                        dims = [
                            [P01 + ng, 128],
                            [ng * NW0, ngrp],
                            [1, ng * D],
                        ]
                        off = sap.offset + kb * hc * NW0
                    else:
                        sap = scr2[:]
                        dims = [
                            [hc * NW2 + ng, 64],
                            [ng * NW2, ngrp],
                            [1, ng * D],
                        ]
                        off = sap.offset
                    diag = AP(tensor=sap.tensor, offset=off, ap=dims)
                    nc.scalar.dma_start(sb[:], diag)
                    sbig.append(sb)
                return sbig

            def emit_back(b, ci, sbig):
                h0 = ci * hc
                ncop = 0
                for gi in range(hc // ng):
                    # bf16 assembly + casting SWDGE store: the band is already
                    # bf16-quantized (sbig), so this loses no precision, makes
                    # the t_ps copies 2x-rate, and keeps the store's
                    # descriptor-gen off the slow sync HWDGE path.
                    asm = asm_pool.tile(
                        [ng * D, W], BF16, tag=f"as{gi}", name=f"as{gi}_{b}_{ci}"
                    )
                    for kb, (w0, M) in enumerate(WBLOCKS):
                        t_ps = psum_t.tile(
                            [ng * D, 128],
                            BF16,
                            tag="t_ps",
                            name=f"t_ps_{b}_{ci}_{gi}_{kb}",
                        )
                        nc.tensor.matmul(
                            t_ps[:, 0:M],
                            sbig[kb][0:M, gi * ng * D : (gi + 1) * ng * D],
                            identity[0:M, 0:M],
                            start=True,
                            stop=True,
                            is_transpose=True,
                        )
                        dst = asm[:, w0 : w0 + M]
                        if ncop % 5 < 3:
                            nc.vector.tensor_copy(dst, t_ps[:, 0:M])
                        else:
                            nc.scalar.copy(dst, t_ps[:, 0:M])
                        ncop += 1
                    # Partition p = d*ng + q -> out[b, d, h0+gi*ng+q, :].
                    ob = oute[b, 0:D, h0 + gi * ng : h0 + gi * ng + ng, :]
                    nc.gpsimd.dma_start(ob, asm[:])

            prev = None
            for b in range(b_dim):
                for ci in range(nchunks):
                    sbig = emit_front(b, ci)
                    if prev is not None:
                        emit_back(*prev)
                    prev = (b, ci, sbig)
            emit_back(*prev)

    nc.finalize()
    return nc


_compiled = {}


def _get_kernel(b_dim, hs):
    key = (b_dim, hs)
    if key not in _compiled:
        _compiled[key] = build_kernel(b_dim, hs)
    return _compiled[key]


def kernel(x_1: np.ndarray, x_2: np.ndarray) -> np.ndarray:
    assert x_1.shape == (B, C, H, W) and x_2.shape == (B, C, H, W)
    x_1 = np.ascontiguousarray(x_1, dtype=np.float32)
    x_2 = np.ascontiguousarray(x_2, dtype=np.float32)
    nc = _get_kernel(B, HS)
    in_maps = [
        {
            "x1": np.ascontiguousarray(x_1[:, :, i * HS : (i + 1) * HS, :]),
            "x2": np.ascontiguousarray(x_2[:, :, i * HS : (i + 1) * HS, :]),
        }
        for i in range(N_CORES)
    ]
    res = run_bass_kernel_spmd(nc, in_maps, core_ids=list(range(N_CORES))).results
    out = np.concatenate([res[i]["out"] for i in range(N_CORES)], axis=2)
    return out
